# revision 29
# baseline (speedup 1.0000x reference)
"""Trainium2 Bass kernel for nn_Evolution_4664334483942 (moe_routing).

Model: per-token relation-specific linear (MoE dispatch) feeding a packed
variable-length-sequence LSTM.

Strategy (data-parallel over sequences, 8 cores, no collectives):
  - Global batch b (0..1023) assigned to core b % 8.  Every core then holds
    128 sequences with lengths 128,127,...,1 (identical structure on every
    core), 8256 tokens each.
  - Host folds W_ih @ W_rel[r].T into per-relation fused weights so the MoE
    projection and the LSTM input projection collapse into ONE GEMM:
        gx[n] = x[n] @ Wfuse[rel_n].T + (W_ih b_rel[rel_n] + b_ih + b_hh)
  - Phase 1 (device): dense bf16 GEMM over rel-sorted 128-token tiles,
    writing gx to DRAM.  Tokens are grouped into time-range CHUNKS with
    per-(chunk, rel) tile capacities computed from the actual data (max
    over cores), so most phase-1 tiles can be emitted INSIDE the LSTM loop
    in bank-granules (4 matmuls each) that fill the tensor engine's
    chain-tail idle gaps.  This both hides phase-1 work and keeps the PE
    out of its low p-state (the PE drops to 1.2 GHz after any idle gap and
    needs ~3us of continuous work to return to 2.4 GHz).
  - Phase 2 (device): 128 sequential LSTM steps.  Each step gathers its
    gx rows via indirect DMA (per-core index table = data, so the SPMD
    instruction stream stays core-independent), feeds them into the gates
    PSUM via an identity matmul, accumulates h @ W_hh.T on top, applies
    sigmoid/tanh on ScalarE, c/h updates on VectorE, PE-transposes h for
    the next step, and streams h out to DRAM (contiguous rows).
"""

import numpy as np
from ml_dtypes import bfloat16

import concourse.bass as bass
import concourse.mybir as mybir
import concourse.tile as tile
from concourse import bass_utils
from concourse.masks import make_identity
from bass_rust import add_dep_helper
from concourse.vector_clock import ScopedClock

F32 = mybir.dt.float32
F32R = mybir.dt.float32r
BF16 = mybir.dt.bfloat16
I32 = mybir.dt.int32
AF = mybir.ActivationFunctionType

NCORES = 8

# Problem constants (hardcoded; kernel.py must be self-contained).
D = 512          # hidden dim
R = 8            # relations
T = 128          # max sequence length / LSTM steps
B = 1024         # global sequences
KD = D // 128    # contraction k-tiles
G = 4 * D        # gate width (2048)
NJB = G // 512   # psum banks for gates

# phase-1 time-chunk boundaries; per-(chunk, rel) tile capacities are
# computed from the actual data at runtime (max over cores).
CHUNK_BOUNDS = (0, 32, 64, 96, 128)
# steady-state JIT emission rate (granules per LSTM step) used to size the
# upfront prologue;  a granule is one (tile, psum-bank) quartet of matmuls.
JIT_RATE = 2
GATHER_AHEAD = 3

# Results of the last device run (test harness reads exec_time_ns from here).
LAST_RESULTS = None


# ---------------------------------------------------------------------------
# Walrus in this toolchain accepts only ONE sync-wait command per instruction;
# Tile's wait assignment can attach several.  Peel the extras onto same-engine
# NOPs placed immediately before the offending instruction.
# ---------------------------------------------------------------------------
def _split_waits_in_list(nc, insts, max_waits=1):
    out = []
    for inst in insts:
        si = inst.sync_info
        if si is not None and si.on_wait is not None and len(si.on_wait) > max_waits:
            waits = list(si.on_wait)
            for w in waits[max_waits:]:
                nop = mybir.InstNoOp(
                    name=nc.get_next_instruction_name(), ins=[], outs=[],
                )
                nop.engine = inst.engine
                nop.sync_info = mybir.SyncInfo(on_wait=[w], on_update=[])
                out.append(nop)
            inst.sync_info = mybir.SyncInfo(
                on_wait=waits[:max_waits], on_update=list(si.on_update or [])
            )
        out.append(inst)
    return out


class PatchedTileContext(tile.TileContext):
    def _lower_ordered_insts(self, ordered):
        for bb_name in list(ordered.keys()):
            ordered[bb_name] = _split_waits_in_list(self.nc, ordered[bb_name])
        super()._lower_ordered_insts(ordered)

    def _drain_and_barrier(self, tick_clock, wait_clock):
        nop_inst = self.nc.sync.nop()
        wait_clock.add_sem_waits(
            nop_inst.ins, ScopedClock({None: tick_clock.global_clock})
        )
        si = nop_inst.ins.sync_info
        if si is not None and si.on_wait and len(si.on_wait) > 1:
            waits = list(si.on_wait)
            nop_inst.ins.sync_info = mybir.SyncInfo(
                on_wait=[waits[0]], on_update=list(si.on_update or [])
            )
            for w in waits[1:]:
                extra = self.nc.sync.nop()
                extra.ins.sync_info = mybir.SyncInfo(on_wait=[w], on_update=[])
        self.nc.sync.drain()
        self.nc.all_engine_barrier()
        assert self.sems is not None
        popped = self.nc._tile_sem_poison_stack.pop()
        assert popped is self._sem_poison
        self.nc.clear_and_free_semaphores(list(self.sems.allocated().values()))
        self.nc.all_engine_barrier()


# ---------------------------------------------------------------------------
# Device program (core-independent instruction stream; per-core variation is
# carried entirely by input data: xt tile contents and the gather index table)
# ---------------------------------------------------------------------------
def build_program(chunks, nsteps=T):
    # chunks: tuple of (t0, t1, caps) with caps = per-rel tile counts
    ntiles = sum(sum(c[2]) for c in chunks)
    nrows = ntiles * 128
    nloc = nsteps * (nsteps + 1) // 2

    # physical tile order: chunk-major, then rel, then tile
    tile_list = []          # (chunk_idx, rel)
    for ci, (_, _, caps) in enumerate(chunks):
        for r in range(R):
            for _ in range(caps[r]):
                tile_list.append((ci, r))
    chunk_base = []
    acc = 0
    for ci, (_, _, caps) in enumerate(chunks):
        chunk_base.append(acc)
        acc += sum(caps)
    chunk_rows_end = [
        (chunk_base[ci] + sum(chunks[ci][2])) * 128 for ci in range(len(chunks))
    ]
    chunk_of_t = {}
    for ci, (t0, t1, _) in enumerate(chunks):
        for t in range(t0, t1):
            chunk_of_t[t] = ci
    # granules needed before the gather of step t can be emitted
    gran_need_at = [0] * (nsteps + 8)
    for t in range(nsteps + 8):
        tt = min(t, nsteps - 1)
        ci = chunk_of_t[tt]
        gran_need_at[t] = (chunk_base[ci] + sum(chunks[ci][2])) * 4
    total_gran = ntiles * 4
    # upfront prologue size so JIT_RATE/step keeps every deadline
    upfront = max(
        gran_need_at[min(t + GATHER_AHEAD + 2, nsteps - 1)] - JIT_RATE * t
        for t in range(nsteps)
    )
    upfront = min(max(upfront, gran_need_at[0]), total_gran)
    # precomputed per-step granule schedule: spread evenly, then push
    # granules earlier where a chunk deadline would be missed
    sched = [0] * nsteps
    jit = total_gran - upfront
    for t in range(nsteps):
        sched[t] = jit // nsteps + (1 if t < jit % nsteps else 0)
    for _ in range(nsteps):
        # forward-fix deadline deficits by stealing from later steps
        moved = False
        cum = upfront
        for t in range(nsteps):
            need = gran_need_at[min(t + GATHER_AHEAD + 1, nsteps - 1)]
            if cum < need:
                deficit = need - cum
                # pull granules from the latest nonzero later slots
                for t2 in range(nsteps - 1, t, -1):
                    while sched[t2] > 0 and deficit > 0 and sched[t] < 3:
                        sched[t2] -= 1
                        sched[t] += 1
                        cum += 1
                        deficit -= 1
                        moved = True
                    if deficit == 0 or sched[t] >= 3:
                        break
            cum += sched[t]
        if not moved:
            break

    nc = bass.Bass(target_bir_lowering=False, debug=False, trn_type="TRN2")

    xt = nc.dram_tensor("xt", [ntiles, 128, KD, 128], BF16, kind="ExternalInput").ap()
    wf = nc.dram_tensor("wf", [R, 128, KD, G], BF16, kind="ExternalInput").ap()
    wh = nc.dram_tensor("wh", [128, KD, G], BF16, kind="ExternalInput").ap()
    brep = nc.dram_tensor("brep", [R, 128, G], BF16, kind="ExternalInput").ap()
    gidx = nc.dram_tensor("gidx", [128, nsteps], I32, kind="ExternalInput").ap()
    out = nc.dram_tensor("out", [nloc, D], BF16, kind="ExternalOutput").ap()
    gx = nc.dram_tensor("gx", [nrows, G], BF16).ap()

    loc_bs = [nsteps - t for t in range(nsteps)]
    loc_off = np.concatenate([[0], np.cumsum(loc_bs)]).astype(int)

    with PatchedTileContext(nc) as tc:
        with tc.tile_pool(name="p1_xt", bufs=5) as xt_pool, \
             tc.tile_pool(name="p1_wf", bufs=4) as wf_pool, \
             tc.tile_pool(name="p1_bi", bufs=3) as bi_pool, \
             tc.tile_pool(name="p1_gx", bufs=4) as gxs_pool, \
             tc.tile_pool(name="p1_ps", bufs=2, space="PSUM") as ps1_pool, \
             tc.tile_pool(name="p2_const", bufs=1) as const_pool, \
             tc.tile_pool(name="p2_gx", bufs=4) as gx_pool, \
             tc.tile_pool(name="p2_act", bufs=2) as act_pool, \
             tc.tile_pool(name="p2_st", bufs=1) as st_pool, \
             tc.tile_pool(name="p2_h", bufs=2) as h_pool, \
             tc.tile_pool(name="p2_ht", bufs=2) as ht_pool, \
             tc.tile_pool(name="p2_ps", bufs=5, space="PSUM") as ps2_pool, \
             tc.tile_pool(name="p2_tr", bufs=1, space="PSUM") as tr_pool:

            # ---------------- phase-1 granule emitter ---------------------
            p1_writes = [[] for _ in chunks]
            wfs = {"cur": None, "cur_r": None, "nxt": None, "nxt_r": None}
            g_state = {"gi": 0, "xt": None, "gxs": None, "xt_pf": None, "pf_i": -1}
            pending_adds = []

            def load_wf(r):
                wf_sb = wf_pool.tile([128, KD, G], BF16, tag="wf_sb")
                nc.sync.dma_start(wf_sb[:, 0:2], wf[r][:, 0:2])
                nc.gpsimd.dma_start(wf_sb[:, 2:4], wf[r][:, 2:4])
                bi_sb = bi_pool.tile([128, G], BF16, tag="bi_sb")
                nc.sync.dma_start(bi_sb[:], brep[r])
                return (wf_sb, bi_sb)

            def ensure_wf(i):
                r = tile_list[i][1]
                if wfs["cur_r"] != r:
                    if wfs["nxt_r"] == r:
                        wfs["cur"], wfs["cur_r"] = wfs["nxt"], r
                    else:
                        wfs["cur"], wfs["cur_r"] = load_wf(r), r
                    wfs["nxt"], wfs["nxt_r"] = None, None
                if wfs["nxt"] is None:
                    for j in range(i + 1, ntiles):
                        r2 = tile_list[j][1]
                        if r2 != wfs["cur_r"]:
                            wfs["nxt"], wfs["nxt_r"] = load_wf(r2), r2
                            break

            def emit_granule(defer=False):
                gi = g_state["gi"]
                if gi >= total_gran:
                    return False
                i, jb = divmod(gi, 4)
                ci = tile_list[i][0]
                if jb == 0:
                    ensure_wf(i)
                    if g_state["pf_i"] == i:
                        g_state["xt"] = g_state["xt_pf"]
                    else:
                        xt_sb = xt_pool.tile([128, KD, 128], BF16, tag="xt_sb", name="xt_sb")
                        nc.sync.dma_start(xt_sb[:], xt[i])
                        g_state["xt"] = xt_sb
                    if i + 1 < ntiles:
                        xt_pf = xt_pool.tile([128, KD, 128], BF16, tag="xt_sb", name="xt_pf")
                        nc.sync.dma_start(xt_pf[:], xt[i + 1])
                        g_state["xt_pf"], g_state["pf_i"] = xt_pf, i + 1
                    g_state["gxs"] = gxs_pool.tile([128, G], BF16, tag="gxs", name="gxs")
                xt_sb, gxs = g_state["xt"], g_state["gxs"]
                wf_sb, bi_sb = wfs["cur"][0], wfs["cur"][1]
                sl = slice(jb * 512, (jb + 1) * 512)
                ps = ps1_pool.tile([128, 512], F32, tag="ps1")
                for k in range(KD):
                    nc.tensor.matmul(
                        ps[:], xt_sb[:, k, :], wf_sb[:, k, sl],
                        start=(k == 0), stop=(k == KD - 1),
                    )

                def do_add(ps=ps, gxs=gxs, sl=sl, bi_sb=bi_sb):
                    nc.vector.tensor_add(gxs[:, sl], ps[:], bi_sb[:, sl])

                if defer:
                    pending_adds.append(do_add)
                else:
                    do_add()
                if jb == 3:
                    def do_write(i=i, gxs=gxs, ci=ci):
                        wi = nc.sync.dma_start(
                            gx[i * 128:(i + 1) * 128, :], gxs[:]
                        )
                        p1_writes[ci].append(wi.ins)
                    if defer:
                        pending_adds.append(do_write)
                    else:
                        do_write()
                g_state["gi"] = gi + 1
                return True

            def flush_adds():
                for f in pending_adds:
                    f()
                pending_adds.clear()

            # keep-warm matmuls: the PE drops to 1.2 GHz after any idle gap,
            # so fill chain-tail gaps with throwaway matmuls when no real
            # phase-1 granules remain to be emitted there
            warm_state = {"ps": None}

            def emit_warm(n):
                for _ in range(n):
                    ps_d = ps1_pool.tile([128, 512], F32, tag="ps1", name="ps_d")
                    nc.tensor.matmul(
                        ps_d[:], ident_b[:], wh_sb[:, 0, 0:512],
                        start=True, stop=True,
                    )

            # upfront prologue
            for _ in range(upfront):
                emit_granule()

            # ---------------- phase 2: LSTM ------------------------------
            wh_sb = const_pool.tile([128, KD, G], BF16)
            nc.sync.dma_start(wh_sb[:], wh[:])
            idx_sb = const_pool.tile([128, nsteps], I32)
            nc.sync.dma_start(idx_sb[:], gidx[:])
            ident = const_pool.tile([128, 128], F32)
            make_identity(nc, ident[:])
            ident_b = const_pool.tile([128, 128], BF16)
            nc.vector.tensor_copy(ident_b[:], ident[:])

            c_sb = st_pool.tile([128, D], F32)
            tmp1 = st_pool.tile([128, D], F32)
            tmp2 = st_pool.tile([128, D], F32)

            ht_sb = None
            gxt_tiles = {}
            banks = {}

            def emit_gather(t):
                ci = chunk_of_t[t]
                # safety net: force-finish this chunk's granules
                while g_state["gi"] < gran_need_at[t]:
                    emit_granule()
                flush_adds()
                bs_t = max(nsteps - t, 2)
                gxt = gx_pool.tile([128, G], BF16, tag="gxt")
                rows_end = chunk_rows_end[ci]
                gi = nc.gpsimd.indirect_dma_start(
                    out=gxt[:bs_t, :],
                    out_offset=None,
                    in_=gx[0:rows_end, :],
                    in_offset=bass.IndirectOffsetOnAxis(
                        ap=idx_sb[:bs_t, t:t + 1], axis=0
                    ),
                )
                # the tracker cannot see through the dynamic row offsets, so
                # order the gather after every write of its chunk explicitly
                for w in p1_writes[ci]:
                    add_dep_helper(gi.ins, w, reason="gather waits gx chunk")
                gxt_tiles[t] = gxt

            def emit_ident(t, jb):
                # first write of bank jb for step t: gates <- gx rows
                bs_t = max(nsteps - t, 2)
                psb = ps2_pool.tile([128, 512], F32, tag="ps2")
                nc.tensor.matmul(
                    psb[:], ident_b[:bs_t, :],
                    gxt_tiles[t][:bs_t, jb * 512:(jb + 1) * 512],
                    start=True, stop=(t == 0),
                )
                banks[(t, jb)] = psb

            # bank processing order: g first so the c-chain overlaps later
            # banks; f before i so tmp1 can start before tmp2
            BORD = (2, 1, 0, 3)
            for tt in range(min(GATHER_AHEAD, nsteps)):
                emit_gather(tt)
            for jb in BORD:
                emit_ident(0, jb)
            for t in range(nsteps):
                bs = nsteps - t
                if t + GATHER_AHEAD < nsteps:
                    emit_gather(t + GATHER_AHEAD)
                owed = sched[t]
                sif = act_pool.tile([128, 2 * D], F32, tag="sif")
                tg = act_pool.tile([128, D], F32, tag="tg")
                so = act_pool.tile([128, D], F32, tag="so")
                act_of = {
                    2: (tg[:], AF.Tanh),
                    0: (sif[:, 0:D], AF.Sigmoid),
                    1: (sif[:, D:2 * D], AF.Sigmoid),
                    3: (so[:], AF.Sigmoid),
                }
                # recurrent accumulation: consume hT half-by-half (k 0,1 then
                # 2,3) so it pipelines with the previous step's tail
                if t > 0:
                    for ks in ((0, 1), (2, 3)):
                        for jb in BORD:
                            sl = slice(jb * 512, (jb + 1) * 512)
                            psb = banks[(t, jb)]
                            for k in ks:
                                nc.tensor.matmul(
                                    psb[:],
                                    ht_sb[:, k * 128:(k + 1) * 128],
                                    wh_sb[:, k, sl],
                                    start=False,
                                    stop=(k == KD - 1),
                                )
                            if ks[0] == 2:
                                dst, fn = act_of[jb]
                                psb_done = banks.pop((t, jb))
                                if jb == 0:
                                    nc.scalar.activation(
                                        dst[:, 0:D // 2],
                                        psb_done[:, 0:D // 2], fn)
                                    nc.scalar.activation(
                                        dst[:, D // 2:D],
                                        psb_done[:, D // 2:D], fn)
                                else:
                                    nc.scalar.activation(dst, psb_done[:], fn)
                                if t + 1 < nsteps:
                                    emit_ident(t + 1, jb)
                else:
                    for jb in BORD:
                        dst, fn = act_of[jb]
                        nc.scalar.activation(dst, banks.pop((t, jb))[:], fn)
                        emit_ident(t + 1, jb)

                # one granule in the PE gap between the ident matmuls and the
                # h transposes (its DVE add is deferred past the c/h chain)
                if owed > 0 and emit_granule(defer=True):
                    owed -= 1

                # c update (full width; overlaps the matmul burst)
                if t == 0:
                    nc.vector.tensor_tensor(
                        c_sb[:], sif[:, 0:D], tg[:], mybir.AluOpType.mult
                    )
                else:
                    for q in (slice(0, D // 2), slice(D // 2, D)):
                        qf = slice(D + q.start, D + q.stop)
                        nc.vector.tensor_tensor(
                            tmp1[:, q], sif[:, qf], c_sb[:, q],
                            mybir.AluOpType.mult
                        )
                        nc.vector.tensor_tensor(
                            tmp2[:, q], sif[:, q], tg[:, q],
                            mybir.AluOpType.mult
                        )
                        nc.vector.tensor_add(c_sb[:, q], tmp1[:, q], tmp2[:, q])
                tc_sb = act_pool.tile([128, D], F32, tag="tc_sb")
                if t == 0:
                    nc.scalar.activation(tc_sb[:], c_sb[:], AF.Tanh)
                else:
                    nc.scalar.activation(
                        tc_sb[:, 0:D // 2], c_sb[:, 0:D // 2], AF.Tanh)
                    nc.scalar.activation(
                        tc_sb[:, D // 2:D], c_sb[:, D // 2:D], AF.Tanh)
                # h / transpose / cast in halves so the next step's first
                # recurrent matmuls (k=0,1) start as soon as half 0 is ready
                h_sb = h_pool.tile([128, D], BF16, tag="h_sb")
                if t < nsteps - 1:
                    trp = tr_pool.tile([128, D], BF16, tag="trp")
                    new_ht = ht_pool.tile([128, D], BF16, tag="ht_sb")
                H = D // 2
                for hh in range(2):
                    sl = slice(hh * H, (hh + 1) * H)
                    nc.vector.tensor_tensor(
                        h_sb[:, sl], so[:, sl], tc_sb[:, sl],
                        mybir.AluOpType.mult,
                    )
                    if t < nsteps - 1:
                        for k in (2 * hh, 2 * hh + 1):
                            nc.tensor.transpose(
                                trp[:, k * 128:(k + 1) * 128],
                                h_sb[:, k * 128:(k + 1) * 128],
                                ident_b[:],
                            )
                        nc.scalar.copy(new_ht[:, sl], trp[:, sl])
                if t < nsteps - 1:
                    ht_sb = new_ht
                # stream out this step's hidden states (packed rows)
                nc.sync.dma_start(
                    out[int(loc_off[t]):int(loc_off[t]) + bs, :], h_sb[:bs, :]
                )
                # remaining granules fill the PE idle at the chain tail
                flush_adds()
                while owed > 0 and emit_granule():
                    owed -= 1
            flush_adds()
            while emit_granule():
                pass
    return nc


# ---------------------------------------------------------------------------
# Host-side data marshaling
# ---------------------------------------------------------------------------
def _expected_layout():
    lengths = T - np.arange(B) // NCORES
    batch_sizes = np.array([(lengths > t).sum() for t in range(T)], dtype=np.int32)
    time_idx = np.concatenate(
        [np.full(bs, t, np.int32) for t, bs in enumerate(batch_sizes)]
    )
    batch_idx = np.concatenate(
        [np.arange(bs, dtype=np.int32) for bs in batch_sizes]
    )
    return batch_sizes, time_idx, batch_idx


def _numpy_reference(embed, W_rel, b_rel, W_ih, W_hh, b_ih, b_hh,
                     nodes, rels, time_idx, batch_idx, batch_sizes):
    """Pure-numpy fallback (only used if the packed layout differs from the
    hardcoded one)."""
    n_steps = int(batch_sizes.shape[0])
    max_bs = int(batch_sizes.max())
    x = embed[nodes]
    y = np.zeros_like(x)
    for r in range(W_rel.shape[0]):
        m = rels == r
        y[m] = x[m] @ W_rel[r].T + b_rel[r]
    d = x.shape[-1]
    xp = np.zeros((n_steps, max_bs, d), x.dtype)
    mask = np.zeros((n_steps, max_bs), bool)
    xp[time_idx, batch_idx] = y
    mask[time_idx, batch_idx] = True
    bias = b_ih + b_hh

    def sig(v):
        return 1.0 / (1.0 + np.exp(-v))

    h = np.zeros((max_bs, d), x.dtype)
    c = np.zeros((max_bs, d), x.dtype)
    hs = np.zeros((n_steps, max_bs, d), x.dtype)
    for t in range(n_steps):
        gates = xp[t] @ W_ih.T + h @ W_hh.T + bias
        i, f, g, o = np.split(gates, 4, axis=-1)
        c_new = sig(f) * c + sig(i) * np.tanh(g)
        h_new = sig(o) * np.tanh(c_new)
        m = mask[t][:, None]
        h = np.where(m, h_new, h)
        c = np.where(m, c_new, c)
        hs[t] = h
    return hs[time_idx, batch_idx]


def _prepare_host(inputs, nsteps=T, bounds=CHUNK_BOUNDS):
    """Build per-core device input dicts + the output unshard map.  Returns
    (in_maps, unshard, chunks) with chunks carrying data-derived per-rel
    tile capacities."""
    embed = np.asarray(inputs["embed"], np.float32)
    W_rel = np.asarray(inputs["W_rel"], np.float32)
    b_rel = np.asarray(inputs["b_rel"], np.float32)
    W_ih = np.asarray(inputs["W_ih"], np.float32)
    W_hh = np.asarray(inputs["W_hh"], np.float32)
    b_ih = np.asarray(inputs["b_ih"], np.float32)
    b_hh = np.asarray(inputs["b_hh"], np.float32)
    nodes = np.asarray(inputs["nodes"])
    rels = np.asarray(inputs["rels"])

    nloc = nsteps * (nsteps + 1) // 2
    nchunks = len(bounds) - 1

    # fused weights & biases (float64 for accuracy, cast down)
    Wfuse = (W_ih.astype(np.float64) @ W_rel.astype(np.float64))
    Wfuse = Wfuse.astype(np.float32)            # [R, G, D]
    btot = (W_ih.astype(np.float64) @ b_rel.astype(np.float64).T).T \
        + (b_ih + b_hh).astype(np.float64)      # [R, G]
    btot = btot.astype(np.float32)

    wf_host = np.ascontiguousarray(
        Wfuse.transpose(0, 2, 1).reshape(R, KD, 128, G).transpose(0, 2, 1, 3)
    ).astype(bfloat16)                           # [R, 128(dk), KD, G]
    wh_host = np.ascontiguousarray(
        W_hh.T.reshape(KD, 128, G).transpose(1, 0, 2)
    ).astype(bfloat16)                           # [128(dk), KD, G]
    brep_host = np.ascontiguousarray(
        np.broadcast_to(btot[:, None, :], (R, 128, G))
    ).astype(bfloat16)

    # local token enumeration (identical structure for every core)
    t_arr = np.concatenate(
        [np.full(nsteps - t, t, np.int64) for t in range(nsteps)]
    )
    j_arr = np.concatenate(
        [np.arange(nsteps - t, dtype=np.int64) for t in range(nsteps)]
    )
    gbs = NCORES * (nsteps - np.arange(nsteps, dtype=np.int64))
    goff = np.concatenate([[0], np.cumsum(gbs)])

    chunk_of_t = np.zeros(nsteps, np.int64)
    for ci in range(nchunks):
        chunk_of_t[bounds[ci]:bounds[ci + 1]] = ci
    ch_loc = chunk_of_t[t_arr]

    # per-core (chunk, rel) token counts -> shared tile capacities
    core_rel = []
    counts = np.zeros((NCORES, nchunks, R), np.int64)
    for core in range(NCORES):
        grow = goff[t_arr] + NCORES * j_arr + core
        rel_loc = rels[grow].astype(np.int64)
        core_rel.append((grow, rel_loc))
        np.add.at(counts[core], (ch_loc, rel_loc), 1)
    caps = np.ceil(counts.max(axis=0) / 128).astype(np.int64)  # [nchunks, R]
    chunks = tuple(
        (int(bounds[ci]), int(bounds[ci + 1]), tuple(int(x) for x in caps[ci]))
        for ci in range(nchunks)
    )
    ntiles = int(caps.sum())

    # segment base row for each (chunk, rel)
    seg_base = np.zeros((nchunks, R), np.int64)
    acc = 0
    for ci in range(nchunks):
        for r in range(R):
            seg_base[ci, r] = acc * 128
            acc += caps[ci, r]

    in_maps = []
    for core in range(NCORES):
        grow, rel_loc = core_rel[core]
        node_loc = nodes[grow]

        order = np.lexsort((j_arr, t_arr, rel_loc, ch_loc))
        key = ch_loc[order] * R + rel_loc[order]
        cnt = np.bincount(key, minlength=nchunks * R)
        if any(cnt[ci * R + r] > caps[ci, r] * 128
               for ci in range(nchunks) for r in range(R)):
            return None  # cannot happen by construction; defensive
        q = np.concatenate([np.arange(c) for c in cnt])
        base_sorted = seg_base[key // R, key % R]
        prow = np.empty(nloc, np.int64)
        prow[order] = base_sorted + q

        gidx_host = np.zeros((128, nsteps), np.int32)
        gidx_host[j_arr, t_arr] = prow

        Xp = np.zeros((ntiles * 128, D), np.float32)
        Xp[prow] = embed[node_loc]
        xt_host = np.ascontiguousarray(
            Xp.reshape(ntiles, 128, KD, 128).transpose(0, 3, 2, 1)
        ).astype(bfloat16)                       # [NT, 128(dk), KD, 128(tok)]

        in_maps.append({
            "xt": xt_host,
            "wf": wf_host,
            "wh": wh_host,
            "brep": brep_host,
            "gidx": gidx_host,
        })

    unshard = {
        "t_arr": t_arr, "j_arr": j_arr, "goff": goff,
        "nloc": nloc,
    }
    return in_maps, unshard, chunks


def kernel(**inputs):
    global LAST_RESULTS
    import os

    # Verify the packed layout matches the hardcoded structure.
    bs_exp, ti_exp, bi_exp = _expected_layout()
    ok = (
        np.array_equal(np.asarray(inputs["batch_sizes"]), bs_exp)
        and np.array_equal(np.asarray(inputs["time_idx"]), ti_exp)
        and np.array_equal(np.asarray(inputs["batch_idx"]), bi_exp)
        and np.asarray(inputs["embed"]).shape == (50000, D)
    )
    if not ok:
        return _numpy_reference(**{k: np.asarray(v) for k, v in inputs.items()})

    prep = _prepare_host(inputs)
    if prep is None:
        return _numpy_reference(**{k: np.asarray(v) for k, v in inputs.items()})
    in_maps, unshard, chunks = prep

    nc = build_program(chunks)
    trace = bool(os.environ.get("KERNEL_TRACE"))
    res = bass_utils.run_bass_kernel_spmd(
        nc, in_maps, core_ids=list(range(NCORES)), trace=trace,
    )
    LAST_RESULTS = res

    t_arr = unshard["t_arr"]
    j_arr = unshard["j_arr"]
    goff = unshard["goff"]
    out_full = np.zeros((len(np.asarray(inputs["time_idx"])), D), np.float32)
    for core in range(NCORES):
        grow = goff[t_arr] + NCORES * j_arr + core
        out_full[grow] = res.results[core]["out"].astype(np.float32)
    return out_full
